# revision 28
# baseline (speedup 1.0000x reference)
"""Mamba chunk-state kernel for Trainium2 (8 NeuronCores, Bass/Tile).

Computes, for inputs
    B  (b=4, s=8192, g=1, n=128)   f32
    x  (b=4, s=8192, h=32, p=64)   f32
    dt (b=4, h=32, c=32, l=256)    f32
    dA (b=4, h=32, c=32, l=256)    f32
the chunked state update
    states[b,c,h,p,n] = sum_l x[b,c,l,h,p] * scale[b,h,c,l] * B[b,c,l,n]
    scale = exp(dA[...,-1:] - dA) * dt

Sharding: core i handles batch b = i//2 and chunk range (i%2)*16..+16.
Each (b, chunk-range) slice is fully independent -> no collectives.

The kernel is HBM-bandwidth bound (8 cores together sit at ~90% of the
chip's aggregate HBM rate), so bytes are the whole game: x / B / out
move as bf16 and dt/dA as f16 (26.6 MB per core instead of 51 MB f32).
dt/dA use f16 rather than bf16 because dA feeds exp(), which amplifies
bf16's coarse mantissa into ~3% scale error, while f16 keeps it at
~0.4%. bf16 also makes the PE matmuls 4x faster than fp32 (1 vs 4
cycles/row), taking the Tensor engine far off the critical path.
End-to-end rel err vs the f32 reference is ~9e-3 on HW (gate 2e-2).

Layouts are chosen so every DMA runs full-rate (contiguous runs >=
512 B) and the scale multiply runs in the DVE 2x perf mode (all
operands 2-byte, packed last dim):
  - x is host-packed per timestep as [l, p*32+h] (h innermost). The
    scale (per (h,l)) then broadcasts over p with a packed stride-1
    last dim, so xw = x * scale is 2 DVE ops per chunk at 2x rate.
  - B is host-packed 4 chunks per load as [128, 1024]: row r holds the
    [B[l=r], B[l=128+r]] pair for each of 4 chunks (2 KB rows in bf16).
  - states come out of PSUM as [pl*32+h, hp*128+n] (hp*4+pl = p); the
    staging tile DMAs to DRAM fully contiguously (4 KB rows) and the
    host untangles the permutation for free during the f32 upcast.

Per (b,c) chunk on a core:
  - dt/dA arrive host-interleaved, 4 chunks per load ([64, 1024] f16
    tile, 2 KB rows) because few-partition bulk loads ([32, 16KB]) run
    at a fraction of DMA rate and 512 B rows sit at the efficiency knee.
  - scale = exp(dA_last - dA) * dt computed in its natural [h=32,l=256]
    layout (ACT exp with per-partition bias, DVE multiply), then
    PE-transposed to [l, h] and ACT-cast to bf16 ([128, 64] sct tile).
  - xw = x * scale: 2 broadcast DVE tensor_mul ops ([128,2048] each).
  - states[pl*32+h, hp*128+n] = sum_l xw[l, hp-block] B[l, n]: 16
    column-blocks x 2 l-halves = 32 bf16 matmuls accumulating in PSUM
    ([128,512] bank tiles), cast PSUM->SBUF via 3 ACT + 1 DVE copies
    (the DVE copy goes first so the ACT-issued out-DMA never parks on a
    cross-engine wait), one fully-contiguous DMA out per chunk.

Loads are issued from the SP ring and stores from the ACT ring (the two
hardware DGE rings). Issuing via the Pool/SWDGE path looks fine in
CoreSim but takes the real device down with NRT_EXEC_UNIT_UNRECOVERABLE
- do not route DMAs through gpsimd here.
"""

import numpy as np

BATCH, SEQLEN, NGROUPS, DSTATE = 4, 8192, 1, 128
NHEADS, HEADDIM, CHUNK = 32, 64, 256
NCHUNKS = SEQLEN // CHUNK  # 32
NCORES = 8
CPC = (BATCH * NCHUNKS) // NCORES  # 16 chunks per core
HP = NHEADS * HEADDIM  # 2048

_cached_nc = None


def _np_bf16():
    import concourse.mybir as mybir

    return mybir.dt.np(mybir.dt.bfloat16)


def _build_nc(repeat=1):
    import concourse.bacc as bacc
    import concourse.mybir as mybir
    import concourse.tile as tile
    from concourse.masks import make_identity

    f32 = mybir.dt.float32
    bf16 = mybir.dt.bfloat16
    Exp = mybir.ActivationFunctionType.Exp

    nc = bacc.Bacc(
        "TRN2",
        target_bir_lowering=False,
        debug=False,
        num_devices=NCORES,
    )

    x_d = nc.dram_tensor("x_s", [CPC * CHUNK, HP], bf16, kind="ExternalInput").ap()
    # B and dt/dA are packed 4 chunks per DMA row (2 KB rows) — 512 B rows
    # sit right at the DMA efficiency knee, and 4x fewer issues helps the ring.
    b_d = nc.dram_tensor("b_s", [(CPC // 4) * 128, 8 * DSTATE], bf16, kind="ExternalInput").ap()
    f16 = mybir.dt.float16
    # dt and dA interleaved per chunk ([cc, 0:32, l] = dt, [cc, 32:64, l] = dA)
    # so each chunk needs one small wide load instead of a slow upfront
    # few-partition bulk load. f16 (not bf16): dA feeds exp(), which amplifies
    # bf16's coarse mantissa into ~3% scale error; f16's 10 mantissa bits keep
    # it at ~0.4%, indistinguishable from the f32 result end to end.
    m_d = nc.dram_tensor("m_s", [CPC // 4, 2 * NHEADS, 4 * CHUNK], f16, kind="ExternalInput").ap()
    out_d = nc.dram_tensor(
        "out_s", [CPC, 128, HP], bf16, kind="ExternalOutput"
    ).ap()

    with tile.TileContext(nc) as tc:
        with (
            tc.tile_pool(name="const", bufs=1) as const_pool,
            tc.tile_pool(name="meta", bufs=3) as meta_pool,
            tc.tile_pool(name="xin", bufs=6) as x_pool,
            tc.tile_pool(name="bin", bufs=3) as b_pool,
            tc.tile_pool(name="xwp", bufs=4) as xw_pool,
            tc.tile_pool(name="scp", bufs=3) as sc_pool,
            tc.tile_pool(name="stgp", bufs=3) as stg_pool,
            tc.tile_pool(name="pstates", bufs=6, space="PSUM") as ps_pool,
            tc.tile_pool(name="ptrans", bufs=2, space="PSUM") as pt_pool,
        ):
            ident = const_pool.tile([32, 32], f16)
            make_identity(nc, ident)

            bpk4 = None
            mt4 = None
            for cc_rep in range(CPC * repeat):
                cc = cc_rep % CPC
                r0 = cc * CHUNK
                c4 = cc % 4
                # ---- loads (l on partitions; fully contiguous rows) ----
                xh0 = x_pool.tile([128, HP], bf16, name="xh0", tag="xh")
                xh1 = x_pool.tile([128, HP], bf16, name="xh1", tag="xh")
                nc.sync.dma_start(xh0[:], x_d[r0 : r0 + 128, :])
                nc.sync.dma_start(xh1[:], x_d[r0 + 128 : r0 + 256, :])
                # B / meta for 4 chunks per load (2 KB rows)
                if c4 == 0:
                    g = cc // 4
                    bpk4 = b_pool.tile([128, 8 * DSTATE], bf16, name="bpk", tag="bh")
                    nc.sync.dma_start(bpk4[:], b_d[g * 128 : (g + 1) * 128, :])
                    mt4 = meta_pool.tile([2 * NHEADS, 4 * CHUNK], f16, name="mt", tag="mt")
                    nc.sync.dma_start(mt4[:], m_d[g])
                bpk = bpk4[:, c4 * 2 * DSTATE : (c4 + 1) * 2 * DSTATE]
                mt = mt4[:, c4 * CHUNK : (c4 + 1) * CHUNK]

                # ---- scale = exp(dA_last - dA) * dt, in [h, l] layout ----
                dec = sc_pool.tile([NHEADS, CHUNK], f16, name="dec", tag="dec")
                nc.scalar.activation(
                    dec[:],
                    mt[NHEADS:, :],
                    Exp,
                    bias=mt[NHEADS:, CHUNK - 1 : CHUNK],
                    scale=-1.0,
                )
                scl = sc_pool.tile([NHEADS, CHUNK], f16, name="scl", tag="scl")
                nc.vector.tensor_mul(scl[:], dec[:], mt[:NHEADS, :])

                # ---- transpose scale to [l, h]: [32,256] -> [128, 64] ----
                # cols 0:32 = heads for l-half 0, cols 32:64 = l-half 1
                ptr = pt_pool.tile([128, 64], f16, name="ptr", tag="ptr")
                nc.tensor.transpose(ptr[:, 0:32], scl[:, 0:128], ident[:])
                nc.tensor.transpose(ptr[:, 32:64], scl[:, 128:256], ident[:])
                sct = sc_pool.tile([128, 64], bf16, name="sct", tag="sct")
                nc.scalar.copy(sct[:], ptr[:])

                # ---- xw = x * scale, broadcast over p (DVE 2x mode) ----
                xw0 = xw_pool.tile([128, HP], bf16, name="xw0", tag="xw")
                xw1 = xw_pool.tile([128, HP], bf16, name="xw1", tag="xw")
                for half, (xh, xw) in enumerate(((xh0, xw0), (xh1, xw1))):
                    sb = (
                        sct[:, half * 32 : (half + 1) * 32]
                        .rearrange("l h -> l () h")
                        .broadcast_to([128, HEADDIM, NHEADS])
                    )
                    nc.vector.tensor_mul(
                        xw[:].rearrange("l (p h) -> l p h", h=NHEADS),
                        xh[:].rearrange("l (p h) -> l p h", h=NHEADS),
                        sb,
                    )

                # ---- states matmuls + PSUM -> SBUF -> DRAM ----
                stg = stg_pool.tile([128, HP], bf16, name="stg", tag="stg")
                for q in range(4):
                    st = ps_pool.tile([128, 512], f32, name="st", tag="st")
                    for r in range(4):
                        hp = q * 4 + r
                        w0 = xw0[:, hp * 128 : (hp + 1) * 128]
                        w1 = xw1[:, hp * 128 : (hp + 1) * 128]
                        nc.tensor.matmul(
                            st[:, r * 128 : (r + 1) * 128], w0, bpk[:, 0:DSTATE],
                            start=True, stop=False,
                        )
                        nc.tensor.matmul(
                            st[:, r * 128 : (r + 1) * 128], w1, bpk[:, DSTATE : 2 * DSTATE],
                            start=False, stop=True,
                        )
                    if q > 0:
                        nc.scalar.copy(stg[:, q * 512 : (q + 1) * 512], st[:])
                    else:
                        nc.vector.tensor_copy(stg[:, q * 512 : (q + 1) * 512], st[:])

                # stg rows are already the DRAM layout: fully contiguous store
                nc.scalar.dma_start(out_d[cc], stg[:])

    nc.compile()
    return nc


def _get_nc():
    global _cached_nc
    if _cached_nc is None:
        _cached_nc = _build_nc()
    return _cached_nc


def _in_maps(B, x, dt, dA_cumsum):
    bf16 = _np_bf16()
    B = np.asarray(B, dtype=np.float32)
    x = np.asarray(x, dtype=np.float32)
    dt = np.asarray(dt, dtype=np.float32)
    dA = np.asarray(dA_cumsum, dtype=np.float32)
    maps = []
    for core in range(NCORES):
        b = core // 2
        c0 = (core % 2) * CPC
        s0, s1 = c0 * CHUNK, (c0 + CPC) * CHUNK
        # x: [s, h, p] -> [s, p*32+h] (h innermost), bf16
        xs = np.ascontiguousarray(
            x[b, s0:s1].astype(bf16).transpose(0, 2, 1)
        ).reshape(CPC * CHUNK, HP)
        # B: [s, n] -> per chunk [128, 256]: row r = [B[l=r], B[l=128+r]]
        bs = np.ascontiguousarray(
            B[b, s0:s1, 0, :].astype(bf16).reshape(CPC // 4, 4, 2, 128, DSTATE)
            .transpose(0, 3, 1, 2, 4)
        ).reshape((CPC // 4) * 128, 8 * DSTATE)
        # meta: per chunk, dt rows then dA rows: [cc, 0:32, l]=dt, [cc, 32:64, l]=dA
        ms = np.empty((CPC, 2 * NHEADS, CHUNK), np.float16)
        ms[:, :NHEADS, :] = dt[b, :, c0 : c0 + CPC, :].transpose(1, 0, 2)
        ms[:, NHEADS:, :] = dA[b, :, c0 : c0 + CPC, :].transpose(1, 0, 2)
        ms = np.ascontiguousarray(
            ms.reshape(CPC // 4, 4, 2 * NHEADS, CHUNK).transpose(0, 2, 1, 3)
        ).reshape(CPC // 4, 2 * NHEADS, 4 * CHUNK)
        maps.append({"x_s": xs, "b_s": bs, "m_s": ms})
    return maps


def _assemble(results):
    out = np.empty((BATCH, NCHUNKS, NHEADS, HEADDIM, DSTATE), np.float32)
    for core in range(NCORES):
        b = core // 2
        c0 = (core % 2) * CPC
        o = np.asarray(results[core]["out_s"]).astype(np.float32)
        # o[cc, pl*32+h, hp*128+n] -> out[cc, h, hp*4+pl, n]
        o = o.reshape(CPC, 4, NHEADS, 16, DSTATE).transpose(0, 2, 3, 1, 4)
        out[b, c0 : c0 + CPC] = o.reshape(CPC, NHEADS, HEADDIM, DSTATE)
    return out


def _run(B, x, dt, dA_cumsum, **run_kwargs):
    from concourse import bass_utils

    nc = _get_nc()
    res = bass_utils.run_bass_kernel_spmd(
        nc, _in_maps(B, x, dt, dA_cumsum), core_ids=list(range(NCORES)), **run_kwargs
    )
    return _assemble(res.results), res


def kernel(B, x, dt, dA_cumsum):
    out, _ = _run(B, x, dt, dA_cumsum)
    return out
